# revision 31
# baseline (speedup 1.0000x reference)
"""AdaptiveCenterLoss on 8 TRN2 NeuronCores.

loss = sum((data - cen[labels])**2) / BATCH

The embedding lookup is computed as a windowed one-hot matmul (the
classic embeddings = onehot(labels) @ table formulation), which is both
exact and fast on TRN2 -- unlike SWDGE indirect-DMA gathers, which cost
~1.1us of GPSIMD ucode per 128 gathered rows (8192 rows/core => ~70us)
and, with multi-offset access patterns, silently degrade to fetching
contiguous rows from the first offset.

Plan:
 - Host sorts all rows by label; core c takes the 8192-row sorted slice
   [8192c, 8192(c+1)).  A tile of 128 consecutive sorted rows spans
   < W*128 consecutive table rows (W=2 for uniform labels; W is chosen
   per input and the graph is cached per W).
 - For each tile the host uploads (fp8): the W*128-row table window,
   the negated data rows, and the one-hot selection matrices
   sel_h[p, i] = (labels_local[i] == 128h + p).
 - On device, TensorE computes per tile
     PSUM = sum_h sel_h^T . W_h  +  I^T . (-data)  =  center - data
   exactly in fp32 (one-hot rows select fp8 table rows bit-exactly).
 - ACT squares PSUM with its running accumulator; per-partition partials
   DMA out and the host sums them.  2e-2 rel-err budget dwarfs the
   ~0.15% bias from fp8-quantised inputs (the diff itself is fp32).

All traffic is contiguous HWDGE DMA: data 2.1MB + windows 4.2MB +
sel 2.1MB = 8.4MB/core.  PSUM is split into two 4-bank halves that
ping-pong between TensorE (writing groups of 8 tiles) and ACT (reading
2048-col spans with one accumulate per group).
"""

import os

import numpy as np

BATCH = 65536
DIM = 256
NUM_CLASSES = 100000
N_CORES = 8
B_CORE = BATCH // N_CORES  # 8192

P = 128                    # SBUF partitions / rows per tile
NT = B_CORE // P           # 64 tiles per core
GROUP = 8                  # tiles per PSUM half (4 banks = 2048 f32)
NG = NT // GROUP           # 8 groups

_cached = {}


def _build_graph(W):
    """W = window size in units of 128 table rows (>=2)."""
    from concourse import bacc, mybir, tile

    nc = bacc.Bacc(
        "TRN2",
        target_bir_lowering=False,
        debug=False,
        num_devices=N_CORES,
    )
    f32 = mybir.dt.float32
    f8 = mybir.dt.float8e4
    WD = W * P             # table rows per window

    CPC = GROUP * (W * DIM + WD + DIM)  # cols per chunk bundle (1 group)
    bund_t = nc.dram_tensor("bundle", [P, 8 * CPC], f8, kind="ExternalInput")
    id_t = nc.dram_tensor("ident", [P, P], f8, kind="ExternalInput")
    out_t = nc.dram_tensor("out", [P, NG + 2], f32, kind="ExternalOutput")

    with tile.TileContext(nc) as tc:
        with tc.tile_pool(name="sb", bufs=1) as sb, \
             tc.tile_pool(name="ps", bufs=1, space="PSUM") as ps:
            ident = sb.tile([P, P], f8)
            bias = sb.tile([P, 1], f32)
            parts = sb.tile([P, NG + 2], f32)
            # 1 group per chunk: single-trigger bundles make fine
            # certification cheap; each group starts on its own 1.05MB
            CH_GROUPS = [1] * 8
            ch_of, ch_off = [], []
            for k, n in enumerate(CH_GROUPS):
                for j in range(n):
                    ch_of.append(k)
                    ch_off.append(j)
            # per-chunk bundle [ctab | sel | negd]: one DMA per chunk,
            # same byte order on the wire as the separate transfers had
            bund = [sb.tile([P, CPC], f8, name=f"bund{k}")
                    for k in range(7)]
            bundh = [sb.tile([P, CPC // 2], f8, name=f"bundh{k}")
                     for k in range(2)]
            CT0, SL0, ND0 = 0, GROUP * W * DIM, GROUP * (W * DIM + WD)
            HG = GROUP // 2
            CT0h, SL0h, ND0h = 0, HG * W * DIM, HG * (W * DIM + WD)
            scratch = [sb.tile([P, GROUP * DIM], f8, name=f"scr{k}")
                       for k in range(2)]
            psum = [ps.tile([P, GROUP * DIM], dtype=f32, space="PSUM",
                            name=f"psum{k}")
                    for k in range(2)]

            nc.sync.dma_start(out=ident[:], in_=id_t.ap()[:])
            nc.vector.memset(bias[:], 0.0)
            for k in range(7):
                nc.sync.dma_start(
                    out=bund[k][:],
                    in_=bund_t.ap()[:, k * CPC:(k + 1) * CPC],
                )
            for hf in range(2):
                nc.sync.dma_start(
                    out=bundh[hf][:],
                    in_=bund_t.ap()[:, 7 * CPC + hf * (CPC // 2):
                                    7 * CPC + (hf + 1) * (CPC // 2)],
                )

            dr = mybir.MatmulPerfMode.DoubleRow if W == 2 else None

            def sel_ap(g, j):
                if g < 7:
                    sbase = j * WD
                    return bund[g][:, SL0 + sbase:SL0 + sbase + WD]
                hf, jj = j // HG, j % HG
                sbase = jj * WD
                return bundh[hf][:, SL0h + sbase:SL0h + sbase + WD]

            def ctab_ap(g, j):
                if g < 7:
                    cbase = j * W * DIM
                    return bund[g][:, CT0 + cbase:CT0 + cbase + W * DIM]
                hf, jj = j // HG, j % HG
                cbase = jj * W * DIM
                return bundh[hf][:, CT0h + cbase:CT0h + cbase + W * DIM]

            def negd_ap(g, j, w):
                if g < 7:
                    return bund[g][:, ND0 + j * DIM:ND0 + j * DIM + w]
                hf, jj = j // HG, j % HG
                return bundh[hf][:, ND0h + jj * DIM:ND0h + jj * DIM + w]

            for g in range(NG):
                pt = psum[g % 2]
                for pair in range(GROUP // 2):
                    nc.tensor.matmul(
                        out=pt[:, pair * 2 * DIM:(pair + 1) * 2 * DIM],
                        lhsT=ident[:],
                        rhs=negd_ap(g, pair * 2, 2 * DIM),
                        start=True,
                        stop=False,
                    )
                for j in range(GROUP):
                    if dr is not None:
                        nc.tensor.matmul(
                            out=pt[:, j * DIM:(j + 1) * DIM],
                            lhsT=sel_ap(g, j).rearrange("p (h m) -> p h m", h=2),
                            rhs=ctab_ap(g, j).rearrange("p (h n) -> p h n", h=2),
                            start=False,
                            stop=True,
                            perf_mode=dr,
                        )
                    else:
                        for h in range(W):
                            nc.tensor.matmul(
                                out=pt[:, j * DIM:(j + 1) * DIM],
                                lhsT=sel_ap(g, j)[:, h * P:(h + 1) * P],
                                rhs=ctab_ap(g, j)[:, h * DIM:(h + 1) * DIM],
                                start=False,
                                stop=(h == W - 1),
                            )
                spans = ([(0, GROUP)] if g < NG - 1
                         else [(0, 4), (4, 6), (6, 8)])
                for si_, (slo, shi) in enumerate(spans):
                    nc.scalar.activation(
                        scratch[g % 2][:, slo * DIM:shi * DIM],
                        pt[:, slo * DIM:shi * DIM],
                        mybir.ActivationFunctionType.Square,
                        bias=bias[:, :1],
                        accum_out=parts[:, g + si_:g + si_ + 1],
                    )

            nc.sync.dma_start(out=out_t.ap()[:], in_=parts[:])

    nc.compile()
    return nc


def _build_graph_raw(W):
    """Raw-engine version: same dataflow as the tile impl but without
    TileContext prologue/epilogue barriers; explicit counting semaphores.
    Chunk schedule [1,1,2,2,2] groups: small first chunks prime the
    matmul+ACT pipeline early."""
    from contextlib import ExitStack

    from concourse import bacc, bass, mybir

    nc = bacc.Bacc(
        "TRN2",
        target_bir_lowering=False,
        debug=False,
        num_devices=N_CORES,
    )
    f32 = mybir.dt.float32
    f8 = mybir.dt.float8e4
    WD = W * P

    CH_GROUPS = [1, 1, 2, 2, 2]
    assert sum(CH_GROUPS) == NG
    ch_start = np.cumsum([0] + CH_GROUPS)
    chunk_of = []
    for k, n in enumerate(CH_GROUPS):
        chunk_of += [k] * n

    CPC = 2 * GROUP * (W * DIM + WD + DIM)  # cols per chunk bundle
    bund_t = nc.dram_tensor("bundle", [P, 4 * CPC], f8, kind="ExternalInput")
    id_t = nc.dram_tensor("ident", [P, P], f8, kind="ExternalInput")
    bias_t = nc.dram_tensor("biasz", [P, 1], f32, kind="ExternalInput")
    out_t = nc.dram_tensor("out", [P, NG + 3], f32, kind="ExternalOutput")

    negd = nc.alloc_sbuf_tensor("negd_sb", [P, NT * DIM], f8)
    ctab = nc.alloc_sbuf_tensor("ctab_sb", [P, NT * W * DIM], f8)
    sel = nc.alloc_sbuf_tensor("sel_sb", [P, NT * WD], f8)
    ident = nc.alloc_sbuf_tensor("ident_sb", [P, P], f8)
    bias = nc.alloc_sbuf_tensor("bias", [P, 1], f32)
    parts = nc.alloc_sbuf_tensor("parts", [P, NG + 3], f32)
    scratch = nc.alloc_sbuf_tensor("scratch", [P, 2 * GROUP * DIM], f8)
    psum = [
        nc.alloc_psum_tensor(f"ps{k}", [P, GROUP * DIM], f32) for k in range(2)
    ]

    # (group, lo_tile, hi_tile, out col) — last group tapered so the
    # final ACT after the last matmul is short.
    ACT_PLAN = []
    col = 0
    for g in range(NG):
        spans = [(0, GROUP)] if g < NG - 1 else [(0, 4), (4, 6), (6, 7), (7, 8)]
        for lo, hi in spans:
            ACT_PLAN.append((g, lo, hi, col))
            col += 1
    N_ACTS = len(ACT_PLAN)
    ACTS_UPTO = {}
    cnt = 0
    for g in range(NG):
        cnt += sum(1 for (gg, _, _, _) in ACT_PLAN if gg == g)
        ACTS_UPTO[g] = cnt

    dr = mybir.MatmulPerfMode.DoubleRow if W == 2 else None

    with ExitStack() as es:
        block = es.enter_context(nc.Block(no_gpsimd_drain=True))
        id_sem = es.enter_context(nc.semaphore("id_sem"))
        vb_sem = es.enter_context(nc.semaphore("vb_sem"))
        mm_sem = es.enter_context(nc.semaphore("mm_sem"))
        act_sem = es.enter_context(nc.semaphore("act_sem"))
        out_sem = es.enter_context(nc.semaphore("out_sem"))
        ch_sems = [
            es.enter_context(nc.semaphore(f"ch{k}"))
            for k in range(len(CH_GROUPS))
        ]

        @block.sync
        def _(sync: bass.BassEngine):
            sync.dma_start(out=ident.ap()[:], in_=id_t.ap()[:]).then_inc(
                id_sem, 16
            )
            for k, n in enumerate(CH_GROUPS):
                lo, hi = ch_start[k] * GROUP, ch_start[k + 1] * GROUP  # tiles
                sync.dma_start(
                    out=ctab.ap()[:, lo * W * DIM:hi * W * DIM],
                    in_=ctab_t.ap()[:, lo * W * DIM:hi * W * DIM],
                ).then_inc(ch_sems[k], 16)
                sync.dma_start(
                    out=sel.ap()[:, lo * WD:hi * WD],
                    in_=sel_t.ap()[:, lo * WD:hi * WD],
                ).then_inc(ch_sems[k], 16)
                sync.dma_start(
                    out=negd.ap()[:, lo * DIM:hi * DIM],
                    in_=negd_t.ap()[:, lo * DIM:hi * DIM],
                ).then_inc(ch_sems[k], 16)
            sync.wait_ge(act_sem, N_ACTS)
            sync.dma_start(out=out_t.ap()[:], in_=parts.ap()[:]).then_inc(
                out_sem, 16
            )
            sync.wait_ge(out_sem, 16)

        @block.tensor
        def _(tensor: bass.BassEngine):
            tensor.wait_ge(id_sem, 16)
            seen = set()
            for g in range(NG):
                pt = psum[g % 2]
                k = chunk_of[g]
                if k not in seen:
                    seen.add(k)
                    tensor.wait_ge(ch_sems[k], 48)
                if g >= 2:
                    tensor.wait_ge(act_sem, ACTS_UPTO[g - 2])
                for pair in range(GROUP // 2):
                    t0 = g * GROUP + pair * 2
                    tensor.matmul(
                        out=pt.ap()[:, pair * 2 * DIM:(pair + 1) * 2 * DIM],
                        lhsT=ident.ap()[:],
                        rhs=negd.ap()[:, t0 * DIM:(t0 + 2) * DIM],
                        start=True,
                        stop=False,
                    )
                for j in range(GROUP):
                    t = g * GROUP + j
                    mm = None
                    if dr is not None:
                        mm = tensor.matmul(
                            out=pt.ap()[:, j * DIM:(j + 1) * DIM],
                            lhsT=sel.ap()[:, t * WD:(t + 1) * WD].rearrange(
                                "p (h m) -> p h m", h=2),
                            rhs=ctab.ap()[:, t * W * DIM:(t + 1) * W * DIM
                                          ].rearrange("p (h n) -> p h n", h=2),
                            start=False,
                            stop=True,
                            perf_mode=dr,
                        )
                    else:
                        for h in range(W):
                            mm = tensor.matmul(
                                out=pt.ap()[:, j * DIM:(j + 1) * DIM],
                                lhsT=sel.ap()[:, t * WD + h * P:
                                              t * WD + (h + 1) * P],
                                rhs=ctab.ap()[:, (t * W + h) * DIM:
                                              (t * W + h + 1) * DIM],
                                start=False,
                                stop=(h == W - 1),
                            )
                    mm.then_inc(mm_sem, 1)

        @block.scalar
        def _(scalar: bass.BassEngine):
            scalar.dma_start(out=bias.ap()[:], in_=bias_t.ap()[:]).then_inc(
                vb_sem, 16
            )
            scalar.wait_ge(vb_sem, 16)
            for g, lo, hi, col in ACT_PLAN:
                pt = psum[g % 2]
                scalar.wait_ge(mm_sem, g * GROUP + hi)
                scalar.activation(
                    scratch.ap()[:, (g % 2) * GROUP * DIM + lo * DIM:
                                 (g % 2) * GROUP * DIM + hi * DIM],
                    pt.ap()[:, lo * DIM:hi * DIM],
                    mybir.ActivationFunctionType.Square,
                    bias=bias.ap()[:, :1],
                    accum_out=parts.ap()[:, col:col + 1],
                ).then_inc(act_sem, 1)

    nc.compile()
    return nc


def _get_graph(W):
    impl = os.environ.get("ACL_IMPL", "tile")
    key = f"nc{impl}{W}"
    if key not in _cached:
        _cached[key] = (
            _build_graph_raw(W) if impl == "raw" else _build_graph(W)
        )
    return _cached[key]


def _make_in_maps(data, cen, labels):
    import ml_dtypes

    f8 = ml_dtypes.float8_e4m3
    data = np.asarray(data, dtype=np.float32)
    cen = np.asarray(cen, dtype=np.float32)
    labels = np.asarray(labels).astype(np.int64)

    order = np.argsort(labels, kind="stable")
    data_s = (-data[order]).astype(f8)          # negated for the +I matmul
    labs_s = labels[order].astype(np.int64)

    cen8 = cen.astype(f8)
    ident = np.eye(P, dtype=np.float32).astype(f8)

    # window size: smallest W with all tile spans < 128*W (min 2)
    tl = labs_s.reshape(N_CORES * NT, P)
    spans = tl.max(1) - tl.min(1)
    W = max(2, int(spans.max()) // P + 1)
    WD = W * P

    in_maps = []
    for c in range(N_CORES):
        seg = slice(c * B_CORE, (c + 1) * B_CORE)
        lab_t = labs_s[seg].reshape(NT, P)               # [64, 128]
        start = np.minimum(lab_t.min(axis=1), NUM_CLASSES - WD)
        ll = (lab_t - start[:, None]).astype(np.int64)   # [64,128] in [0, WD)
        assert ll.min() >= 0 and ll.max() < WD

        # negdata [128, NT*DIM]: tile t cols, partition p = sorted row 128t+p
        nd = data_s[seg].reshape(NT, P, DIM).transpose(1, 0, 2).reshape(
            P, NT * DIM)

        # ctab [128, NT*W*DIM]: tile t, half h, partition p = cen[start+128h+p]
        idx = start[:, None] + np.arange(WD)[None, :]    # [64, WD]
        rows = cen8[idx]                                 # [64, WD, 256]
        rows = rows.reshape(NT, W, P, DIM).transpose(2, 0, 1, 3)
        ct = np.ascontiguousarray(rows).reshape(P, NT * W * DIM)

        # selin [128, NT*WD]: tile t cols [t*WD, (t+1)*WD): one-hot
        # sel[p, 128h + i] = (ll[t, i] == 128h + p)
        s = np.zeros((NT, P, WD), dtype=np.float32)
        t_i = np.repeat(np.arange(NT), P)
        i_i = np.tile(np.arange(P), NT)
        llf = ll.ravel()
        s[t_i, llf % P, (llf // P) * P + i_i] = 1.0
        si = s.transpose(1, 0, 2).reshape(P, NT * WD).astype(f8)

        nd = np.ascontiguousarray(nd)
        si = np.ascontiguousarray(si)
        # bundle per chunk of 8 tiles ([ctab|sel|negd]); last chunk as
        # two 4-tile half-bundles so the final group's first half prefires
        chunks = []
        bounds = [(k * GROUP, (k + 1) * GROUP) for k in range(7)]
        bounds += [(56, 60), (60, 64)]
        for lo, hi in bounds:
            chunks.append(np.concatenate([
                ct[:, lo * W * DIM:hi * W * DIM],
                si[:, lo * WD:hi * WD],
                nd[:, lo * DIM:hi * DIM],
            ], axis=1))
        in_maps.append({
            "bundle": np.ascontiguousarray(np.concatenate(chunks, axis=1)),
            "ident": ident,
            "biasz": np.zeros((P, 1), dtype=np.float32),
        })
    return W, in_maps


def _run(data, cen, labels, trace=False):
    import time

    from concourse.bass_utils import run_bass_kernel_spmd

    W, in_maps = _make_in_maps(data, cen, labels)
    nc = _get_graph(W)
    last_err = None
    for attempt in range(4):
        try:
            res = run_bass_kernel_spmd(
                nc, in_maps, core_ids=list(range(N_CORES)), trace=trace
            )
        except Exception as e:  # transient NRT device flakes
            last_err = e
            time.sleep(2.0)
            continue
        total = float(
            np.sum(
                [res.results[i]["out"].astype(np.float64) for i in range(N_CORES)]
            )
        )
        if np.isfinite(total):
            return np.float32(total / BATCH), res
    if last_err is not None:
        raise last_err
    return np.float32(total / BATCH), res


def kernel(data, cen, labels):
    out, _ = _run(data, cen, labels)
    return out


# revision 32
# speedup vs baseline: 1.1517x; 1.1517x over previous
"""AdaptiveCenterLoss on 8 TRN2 NeuronCores.

loss = sum((data - cen[labels])**2) / BATCH

The embedding lookup is computed as a windowed one-hot matmul (the
classic embeddings = onehot(labels) @ table formulation), which is both
exact and fast on TRN2 -- unlike SWDGE indirect-DMA gathers, which cost
~1.1us of GPSIMD ucode per 128 gathered rows (8192 rows/core => ~70us)
and, with multi-offset access patterns, silently degrade to fetching
contiguous rows from the first offset.

Plan:
 - Host sorts all rows by label; core c takes the 8192-row sorted slice
   [8192c, 8192(c+1)).  A tile of 128 consecutive sorted rows spans
   < W*128 consecutive table rows (W=2 for uniform labels; W is chosen
   per input and the graph is cached per W).
 - For each tile the host uploads (fp8): the W*128-row table window,
   the negated data rows, and the one-hot selection matrices
   sel_h[p, i] = (labels_local[i] == 128h + p).
 - On device, TensorE computes per tile
     PSUM = sum_h sel_h^T . W_h  +  I^T . (-data)  =  center - data
   exactly in fp32 (one-hot rows select fp8 table rows bit-exactly).
 - ACT squares PSUM with its running accumulator; per-partition partials
   DMA out and the host sums them.  2e-2 rel-err budget dwarfs the
   ~0.15% bias from fp8-quantised inputs (the diff itself is fp32).

All traffic is contiguous HWDGE DMA: data 2.1MB + windows 4.2MB +
sel 2.1MB = 8.4MB/core.  PSUM is split into two 4-bank halves that
ping-pong between TensorE (writing groups of 8 tiles) and ACT (reading
2048-col spans with one accumulate per group).
"""

import os

import numpy as np

BATCH = 65536
DIM = 256
NUM_CLASSES = 100000
N_CORES = 8
B_CORE = BATCH // N_CORES  # 8192

P = 128                    # SBUF partitions / rows per tile
NT = B_CORE // P           # 64 tiles per core
GROUP = 8                  # tiles per PSUM half (4 banks = 2048 f32)
NG = NT // GROUP           # 8 groups

_cached = {}


def _build_graph(W):
    """W = window size in units of 128 table rows (>=2)."""
    from concourse import bacc, mybir, tile

    nc = bacc.Bacc(
        "TRN2",
        target_bir_lowering=False,
        debug=False,
        num_devices=N_CORES,
    )
    f32 = mybir.dt.float32
    f8 = mybir.dt.float8e4
    WD = W * P             # table rows per window

    CPC = GROUP * (W * DIM + WD + DIM)  # cols per chunk bundle (1 group)
    bund_t = nc.dram_tensor("bundle", [P, 8 * CPC], f8, kind="ExternalInput")
    id_t = nc.dram_tensor("ident", [P, P], f8, kind="ExternalInput")
    out_t = nc.dram_tensor("out", [P, NG], f32, kind="ExternalOutput")

    with tile.TileContext(nc) as tc:
        with tc.tile_pool(name="sb", bufs=1) as sb, \
             tc.tile_pool(name="ps", bufs=1, space="PSUM") as ps:
            ident = sb.tile([P, P], f8)
            bias = sb.tile([P, 1], f32)
            parts = sb.tile([P, NG], f32)
            # 1 group per chunk: single-trigger bundles make fine
            # certification cheap; each group starts on its own 1.05MB
            CH_GROUPS = [1] * 8
            ch_of, ch_off = [], []
            for k, n in enumerate(CH_GROUPS):
                for j in range(n):
                    ch_of.append(k)
                    ch_off.append(j)
            # per-chunk bundle [ctab | sel | negd]: one DMA per chunk,
            # same byte order on the wire as the separate transfers had
            bund = [sb.tile([P, CPC], f8, name=f"bund{k}")
                    for k, n in enumerate(CH_GROUPS)]
            CT0, SL0, ND0 = 0, GROUP * W * DIM, GROUP * (W * DIM + WD)
            scratch = [sb.tile([P, GROUP * DIM], f8, name=f"scr{k}")
                       for k in range(2)]
            psum = [ps.tile([P, GROUP * DIM], dtype=f32, space="PSUM",
                            name=f"psum{k}")
                    for k in range(2)]

            nc.sync.dma_start(out=ident[:], in_=id_t.ap()[:])
            nc.vector.memset(bias[:], 0.0)
            for k, n in enumerate(CH_GROUPS):
                nc.sync.dma_start(
                    out=bund[k][:],
                    in_=bund_t.ap()[:, k * CPC:(k + 1) * CPC],
                )

            dr = mybir.MatmulPerfMode.DoubleRow if W == 2 else None
            for g in range(NG):
                pt = psum[g % 2]
                k, go = ch_of[g], ch_off[g] * GROUP  # chunk id, group offset
                # -data for the whole group: identity stays stationary
                # across back-to-back matmuls; 512-col rhs spans 2 tiles
                # (one PSUM bank).
                for pair in range(GROUP // 2):
                    nc.tensor.matmul(
                        out=pt[:, pair * 2 * DIM:(pair + 1) * 2 * DIM],
                        lhsT=ident[:],
                        rhs=bund[k][:, ND0 + (go + pair * 2) * DIM:
                                   ND0 + (go + pair * 2 + 2) * DIM],
                        start=True,
                        stop=False,
                    )
                for j in range(GROUP):
                    sbase = (go + j) * WD
                    cbase = (go + j) * W * DIM
                    if dr is not None:
                        # fp8 DoubleRow: both 256-row window halves in one
                        # matmul; [p, h*X+x] slices are already [p, h, x].
                        nc.tensor.matmul(
                            out=pt[:, j * DIM:(j + 1) * DIM],
                            lhsT=bund[k][:, SL0 + sbase:SL0 + sbase + WD
                                         ].rearrange("p (h m) -> p h m", h=2),
                            rhs=bund[k][:, CT0 + cbase:CT0 + cbase + W * DIM
                                        ].rearrange("p (h n) -> p h n", h=2),
                            start=False,
                            stop=True,
                            perf_mode=dr,
                        )
                    else:
                        for h in range(W):
                            nc.tensor.matmul(
                                out=pt[:, j * DIM:(j + 1) * DIM],
                                lhsT=bund[k][:, SL0 + sbase + h * P:
                                             SL0 + sbase + (h + 1) * P],
                                rhs=bund[k][:, CT0 + cbase + h * DIM:
                                            CT0 + cbase + (h + 1) * DIM],
                                start=False,
                                stop=(h == W - 1),
                            )
                nc.scalar.activation(
                    scratch[g % 2][:],
                    pt[:],
                    mybir.ActivationFunctionType.Square,
                    bias=bias[:, :1],
                    accum_out=parts[:, g:g + 1],
                )

            nc.sync.dma_start(out=out_t.ap()[:], in_=parts[:])

    nc.compile()
    return nc


def _build_graph_raw(W):
    """Raw-engine version: same dataflow as the tile impl but without
    TileContext prologue/epilogue barriers; explicit counting semaphores.
    Chunk schedule [1,1,2,2,2] groups: small first chunks prime the
    matmul+ACT pipeline early."""
    from contextlib import ExitStack

    from concourse import bacc, bass, mybir

    nc = bacc.Bacc(
        "TRN2",
        target_bir_lowering=False,
        debug=False,
        num_devices=N_CORES,
    )
    f32 = mybir.dt.float32
    f8 = mybir.dt.float8e4
    WD = W * P

    CH_GROUPS = [1, 1, 2, 2, 2]
    assert sum(CH_GROUPS) == NG
    ch_start = np.cumsum([0] + CH_GROUPS)
    chunk_of = []
    for k, n in enumerate(CH_GROUPS):
        chunk_of += [k] * n

    CPC = 2 * GROUP * (W * DIM + WD + DIM)  # cols per chunk bundle
    bund_t = nc.dram_tensor("bundle", [P, 4 * CPC], f8, kind="ExternalInput")
    id_t = nc.dram_tensor("ident", [P, P], f8, kind="ExternalInput")
    bias_t = nc.dram_tensor("biasz", [P, 1], f32, kind="ExternalInput")
    out_t = nc.dram_tensor("out", [P, NG + 3], f32, kind="ExternalOutput")

    negd = nc.alloc_sbuf_tensor("negd_sb", [P, NT * DIM], f8)
    ctab = nc.alloc_sbuf_tensor("ctab_sb", [P, NT * W * DIM], f8)
    sel = nc.alloc_sbuf_tensor("sel_sb", [P, NT * WD], f8)
    ident = nc.alloc_sbuf_tensor("ident_sb", [P, P], f8)
    bias = nc.alloc_sbuf_tensor("bias", [P, 1], f32)
    parts = nc.alloc_sbuf_tensor("parts", [P, NG + 3], f32)
    scratch = nc.alloc_sbuf_tensor("scratch", [P, 2 * GROUP * DIM], f8)
    psum = [
        nc.alloc_psum_tensor(f"ps{k}", [P, GROUP * DIM], f32) for k in range(2)
    ]

    # (group, lo_tile, hi_tile, out col) — last group tapered so the
    # final ACT after the last matmul is short.
    ACT_PLAN = []
    col = 0
    for g in range(NG):
        spans = [(0, GROUP)] if g < NG - 1 else [(0, 4), (4, 6), (6, 7), (7, 8)]
        for lo, hi in spans:
            ACT_PLAN.append((g, lo, hi, col))
            col += 1
    N_ACTS = len(ACT_PLAN)
    ACTS_UPTO = {}
    cnt = 0
    for g in range(NG):
        cnt += sum(1 for (gg, _, _, _) in ACT_PLAN if gg == g)
        ACTS_UPTO[g] = cnt

    dr = mybir.MatmulPerfMode.DoubleRow if W == 2 else None

    with ExitStack() as es:
        block = es.enter_context(nc.Block(no_gpsimd_drain=True))
        id_sem = es.enter_context(nc.semaphore("id_sem"))
        vb_sem = es.enter_context(nc.semaphore("vb_sem"))
        mm_sem = es.enter_context(nc.semaphore("mm_sem"))
        act_sem = es.enter_context(nc.semaphore("act_sem"))
        out_sem = es.enter_context(nc.semaphore("out_sem"))
        ch_sems = [
            es.enter_context(nc.semaphore(f"ch{k}"))
            for k in range(len(CH_GROUPS))
        ]

        @block.sync
        def _(sync: bass.BassEngine):
            sync.dma_start(out=ident.ap()[:], in_=id_t.ap()[:]).then_inc(
                id_sem, 16
            )
            for k, n in enumerate(CH_GROUPS):
                lo, hi = ch_start[k] * GROUP, ch_start[k + 1] * GROUP  # tiles
                sync.dma_start(
                    out=ctab.ap()[:, lo * W * DIM:hi * W * DIM],
                    in_=ctab_t.ap()[:, lo * W * DIM:hi * W * DIM],
                ).then_inc(ch_sems[k], 16)
                sync.dma_start(
                    out=sel.ap()[:, lo * WD:hi * WD],
                    in_=sel_t.ap()[:, lo * WD:hi * WD],
                ).then_inc(ch_sems[k], 16)
                sync.dma_start(
                    out=negd.ap()[:, lo * DIM:hi * DIM],
                    in_=negd_t.ap()[:, lo * DIM:hi * DIM],
                ).then_inc(ch_sems[k], 16)
            sync.wait_ge(act_sem, N_ACTS)
            sync.dma_start(out=out_t.ap()[:], in_=parts.ap()[:]).then_inc(
                out_sem, 16
            )
            sync.wait_ge(out_sem, 16)

        @block.tensor
        def _(tensor: bass.BassEngine):
            tensor.wait_ge(id_sem, 16)
            seen = set()
            for g in range(NG):
                pt = psum[g % 2]
                k = chunk_of[g]
                if k not in seen:
                    seen.add(k)
                    tensor.wait_ge(ch_sems[k], 48)
                if g >= 2:
                    tensor.wait_ge(act_sem, ACTS_UPTO[g - 2])
                for pair in range(GROUP // 2):
                    t0 = g * GROUP + pair * 2
                    tensor.matmul(
                        out=pt.ap()[:, pair * 2 * DIM:(pair + 1) * 2 * DIM],
                        lhsT=ident.ap()[:],
                        rhs=negd.ap()[:, t0 * DIM:(t0 + 2) * DIM],
                        start=True,
                        stop=False,
                    )
                for j in range(GROUP):
                    t = g * GROUP + j
                    mm = None
                    if dr is not None:
                        mm = tensor.matmul(
                            out=pt.ap()[:, j * DIM:(j + 1) * DIM],
                            lhsT=sel.ap()[:, t * WD:(t + 1) * WD].rearrange(
                                "p (h m) -> p h m", h=2),
                            rhs=ctab.ap()[:, t * W * DIM:(t + 1) * W * DIM
                                          ].rearrange("p (h n) -> p h n", h=2),
                            start=False,
                            stop=True,
                            perf_mode=dr,
                        )
                    else:
                        for h in range(W):
                            mm = tensor.matmul(
                                out=pt.ap()[:, j * DIM:(j + 1) * DIM],
                                lhsT=sel.ap()[:, t * WD + h * P:
                                              t * WD + (h + 1) * P],
                                rhs=ctab.ap()[:, (t * W + h) * DIM:
                                              (t * W + h + 1) * DIM],
                                start=False,
                                stop=(h == W - 1),
                            )
                    mm.then_inc(mm_sem, 1)

        @block.scalar
        def _(scalar: bass.BassEngine):
            scalar.dma_start(out=bias.ap()[:], in_=bias_t.ap()[:]).then_inc(
                vb_sem, 16
            )
            scalar.wait_ge(vb_sem, 16)
            for g, lo, hi, col in ACT_PLAN:
                pt = psum[g % 2]
                scalar.wait_ge(mm_sem, g * GROUP + hi)
                scalar.activation(
                    scratch.ap()[:, (g % 2) * GROUP * DIM + lo * DIM:
                                 (g % 2) * GROUP * DIM + hi * DIM],
                    pt.ap()[:, lo * DIM:hi * DIM],
                    mybir.ActivationFunctionType.Square,
                    bias=bias.ap()[:, :1],
                    accum_out=parts.ap()[:, col:col + 1],
                ).then_inc(act_sem, 1)

    nc.compile()
    return nc


def _get_graph(W):
    impl = os.environ.get("ACL_IMPL", "tile")
    key = f"nc{impl}{W}"
    if key not in _cached:
        _cached[key] = (
            _build_graph_raw(W) if impl == "raw" else _build_graph(W)
        )
    return _cached[key]


def _make_in_maps(data, cen, labels):
    import ml_dtypes

    f8 = ml_dtypes.float8_e4m3
    data = np.asarray(data, dtype=np.float32)
    cen = np.asarray(cen, dtype=np.float32)
    labels = np.asarray(labels).astype(np.int64)

    order = np.argsort(labels, kind="stable")
    data_s = (-data[order]).astype(f8)          # negated for the +I matmul
    labs_s = labels[order].astype(np.int64)

    cen8 = cen.astype(f8)
    ident = np.eye(P, dtype=np.float32).astype(f8)

    # window size: smallest W with all tile spans < 128*W (min 2)
    tl = labs_s.reshape(N_CORES * NT, P)
    spans = tl.max(1) - tl.min(1)
    W = max(2, int(spans.max()) // P + 1)
    WD = W * P

    in_maps = []
    for c in range(N_CORES):
        seg = slice(c * B_CORE, (c + 1) * B_CORE)
        lab_t = labs_s[seg].reshape(NT, P)               # [64, 128]
        start = np.minimum(lab_t.min(axis=1), NUM_CLASSES - WD)
        ll = (lab_t - start[:, None]).astype(np.int64)   # [64,128] in [0, WD)
        assert ll.min() >= 0 and ll.max() < WD

        # negdata [128, NT*DIM]: tile t cols, partition p = sorted row 128t+p
        nd = data_s[seg].reshape(NT, P, DIM).transpose(1, 0, 2).reshape(
            P, NT * DIM)

        # ctab [128, NT*W*DIM]: tile t, half h, partition p = cen[start+128h+p]
        idx = start[:, None] + np.arange(WD)[None, :]    # [64, WD]
        rows = cen8[idx]                                 # [64, WD, 256]
        rows = rows.reshape(NT, W, P, DIM).transpose(2, 0, 1, 3)
        ct = np.ascontiguousarray(rows).reshape(P, NT * W * DIM)

        # selin [128, NT*WD]: tile t cols [t*WD, (t+1)*WD): one-hot
        # sel[p, 128h + i] = (ll[t, i] == 128h + p)
        s = np.zeros((NT, P, WD), dtype=np.float32)
        t_i = np.repeat(np.arange(NT), P)
        i_i = np.tile(np.arange(P), NT)
        llf = ll.ravel()
        s[t_i, llf % P, (llf // P) * P + i_i] = 1.0
        si = s.transpose(1, 0, 2).reshape(P, NT * WD).astype(f8)

        nd = np.ascontiguousarray(nd)
        si = np.ascontiguousarray(si)
        # bundle per chunk of 8 tiles: [ctab | sel | negd]
        ntc = GROUP
        chunks = []
        for k in range(NT // ntc):
            lo, hi = k * ntc, (k + 1) * ntc
            chunks.append(np.concatenate([
                ct[:, lo * W * DIM:hi * W * DIM],
                si[:, lo * WD:hi * WD],
                nd[:, lo * DIM:hi * DIM],
            ], axis=1))
        in_maps.append({
            "bundle": np.ascontiguousarray(np.concatenate(chunks, axis=1)),
            "ident": ident,
            "biasz": np.zeros((P, 1), dtype=np.float32),
        })
    return W, in_maps


def _run(data, cen, labels, trace=False):
    import time

    from concourse.bass_utils import run_bass_kernel_spmd

    W, in_maps = _make_in_maps(data, cen, labels)
    nc = _get_graph(W)
    last_err = None
    for attempt in range(4):
        try:
            res = run_bass_kernel_spmd(
                nc, in_maps, core_ids=list(range(N_CORES)), trace=trace
            )
        except Exception as e:  # transient NRT device flakes
            last_err = e
            time.sleep(2.0)
            continue
        total = float(
            np.sum(
                [res.results[i]["out"].astype(np.float64) for i in range(N_CORES)]
            )
        )
        if np.isfinite(total):
            return np.float32(total / BATCH), res
    if last_err is not None:
        raise last_err
    return np.float32(total / BATCH), res


def kernel(data, cen, labels):
    out, _ = _run(data, cen, labels)
    return out
